# revision 18
# baseline (speedup 1.0000x reference)
"""TRN2 Bass kernel for nn_Attention_25340307046737.

Multi-head attention (B=2, S=2048, D=1024, H=16, HD=64) with interleaved RoPE,
returning (out, attn_weights).  attn_weights is (2,16,2048,2048) f32 = 512MB,
so the kernel is HBM-write bound; everything is organized around streaming
those writes.

Sharding (8 NeuronCores): core c -> batch b = c//4, head group g = c%4
(4 heads each).  Column-parallel wq/wk/wv, row-parallel wo; the 4 partial
wo outputs per batch are summed on the host during unshard.

Per-core layout strategy:
  - x is passed transposed (D, S); q/k are computed directly in head-major
    (256, S) layout, v in seq-major (S, 256).
  - RoPE pair-swap is folded into the weights: a second projection with
    row-swapped wq/wk gives q_swap "for free" on the PE, then
    q_roped = q*cos_rep + q_swap*sin_sgn  (two DVE muls + add).
  - scores are computed transposed (s2 on partitions, s1 on free axis):
    lhsT = kT tile, rhs = qT tile.  exp via ACT (scale=1/8 folded in).
  - AV matmul uses a ones-augmented v (65 columns): row 64 of the PSUM
    accumulator is the softmax denominator, computed for free.
  - the attention loop is software-pipelined one (head, s1-chunk) unit
    deep: unit i's 16 score matmuls + exps run while unit i-1's AV
    matmuls run, so the PE never waits on ACT instruction-by-instruction
    (keeps the HAM clock gate open).
  - normalization: ACT reciprocal of the sums row, broadcast across
    partitions with a K=1 ones matmul, then 4 chunked in-place DVE
    multiplies each followed by its 1MB DMA of attn^T to HBM.

The host unshards: attn shard (4,2048,2048) holds attn^T per head, so the
host transposes per (b,h) 16MB block; wo partials are summed per batch.
"""
import os
import numpy as np

import concourse.bacc as bacc
import concourse.mybir as mybir
import concourse.tile as tile
from concourse.bass import ds
from concourse.bass_utils import run_bass_kernel_spmd

F32 = mybir.dt.float32
F32R = mybir.dt.float32r
F16 = mybir.dt.float16

B, S, D = 2, 2048, 1024
H, HD = 16, 64
HLOC = 4            # heads per core
EL = HLOC * HD      # 256 local output dims
NCORES = 8
KC = D // 128       # 8 contraction chunks
SC = S // 512       # 4 seq chunks of 512
SM = S // 128       # 16 seq chunks of 128

_CACHE = {}
last_exec_time_ns = None


def _build():
    nc = bacc.Bacc("TRN2", target_bir_lowering=False, debug=False,
                   num_devices=NCORES)

    xt_d = nc.dram_tensor("xt", [D, S], F16, kind="ExternalInput")
    wqt_d = nc.dram_tensor("wqt", [D, EL], F16, kind="ExternalInput")
    wkt_d = nc.dram_tensor("wkt", [D, EL], F16, kind="ExternalInput")
    wvt_d = nc.dram_tensor("wvt", [D, EL], F16, kind="ExternalInput")
    wot_d = nc.dram_tensor("wot", [HLOC, HD, D], F16, kind="ExternalInput")
    cos_d = nc.dram_tensor("cos", [128, S], F32, kind="ExternalInput")
    sin_d = nc.dram_tensor("sin", [128, S], F32, kind="ExternalInput")

    attn_d = nc.dram_tensor("attn", [HLOC, S, S], F16, kind="ExternalOutput")
    outp_d = nc.dram_tensor("out_p", [S, D], F32, kind="ExternalOutput")

    Exp = mybir.ActivationFunctionType.Exp

    with tile.TileContext(nc) as tc:
        with (
            tc.tile_pool(name="persist", bufs=1) as pp,
            tc.tile_pool(name="ps_sc", bufs=1, space="PSUM") as ps_sc,
        ):
            # ---- persistent tiles ----
            qt = [pp.tile([128, S], F16, tag=f"qt{i}", name=f"qt{i}") for i in range(2)]
            kt = [pp.tile([128, S], F16, tag=f"kt{i}", name=f"kt{i}") for i in range(2)]
            v = [pp.tile([128, HLOC, HD + 1], F16, tag=f"v{i}", name=f"v{i}") for i in range(SM)]
            woT = [pp.tile([HD, D], F16, tag=f"wo{h}", name=f"wo{h}") for h in range(HLOC)]
            ones_col = pp.tile([128, 1], F32, tag="ones_col")

            # ---- phase 1: projections + rope ----
            with (
                tc.tile_pool(name="p1", bufs=1) as p1,
                tc.tile_pool(name="psA", bufs=2, space="PSUM") as psA,
            ):
                xt = p1.tile([128, KC, S], F16, tag="xt")
                cos = p1.tile([128, S], F32, tag="cos")
                sin = p1.tile([128, S], F32, tag="sin")
                xt_r = xt_d[:].rearrange("(c p) s -> p c s", p=128)

                with (
                    tc.tile_pool(name="w1", bufs=1) as w1,
                    tc.tile_pool(name="tmp", bufs=2) as tp,
                ):
                    wsb = {}
                    for name in ("wqt", "wkt"):
                        wsb[name] = w1.tile([128, KC, EL], F16, tag=name, name=name)
                    # load order: q weights, first xt column block, then the
                    # rest -- lets the first matmuls start early.
                    nc.sync.dma_start(
                        wsb["wqt"][:], wqt_d[:].rearrange("(c p) e -> p c e", p=128))
                    for sc in range(SC):
                        s_sl = ds(sc * 512, 512)
                        for kc in range(KC):
                            nc.sync.dma_start(xt[:, kc, s_sl], xt_r[:, kc, s_sl])
                        if sc == 0:
                            nc.sync.dma_start(
                                wsb["wkt"][:],
                                wkt_d[:].rearrange("(c p) e -> p c e", p=128))
                            nc.sync.dma_start(cos[:], cos_d[:])
                            nc.sync.dma_start(sin[:], sin_d[:])
                    nc.vector.memset(ones_col[:], 1.0)
                    for i in range(SM):
                        nc.vector.tensor_copy(
                            v[i][:, :, HD:HD + 1],
                            ones_col[:].broadcast_to([128, HLOC, 1]),
                        )
                    for h in range(HLOC):
                        nc.sync.dma_start(woT[h][:], wot_d[h])

                    swap_mask = [i ^ 1 for i in range(32)]
                    for sc in range(SC):
                        s_sl = ds(sc * 512, 512)
                        for dest, wn in ((qt, "wqt"), (kt, "wkt")):
                            for mt in range(2):
                                q_ps = psA.tile([128, 512], F32, tag="pp", name="q_ps", bufs=4)
                                for kc in range(KC):
                                    nc.tensor.matmul(
                                        q_ps[:],
                                        wsb[wn][:, kc, ds(mt * 128, 128)],
                                        xt[:, kc, s_sl],
                                        start=(kc == 0), stop=(kc == KC - 1),
                                    )
                                qsw = tp.tile([128, 512], F32, tag="qsw", name="qsw")
                                nc.vector.stream_shuffle(qsw[:], q_ps[:], swap_mask)
                                t1 = tp.tile([128, 512], F32, tag="t1", name="t1")
                                t2 = tp.tile([128, 512], F32, tag="t2", name="t2")
                                nc.vector.tensor_mul(t1[:], q_ps[:], cos[:, s_sl])
                                nc.vector.tensor_mul(t2[:], qsw[:], sin[:, s_sl])
                                nc.vector.tensor_add(dest[mt][:, s_sl], t1[:], t2[:])

                with tc.tile_pool(name="w2", bufs=1) as w2:
                    wvt = w2.tile([128, KC, EL], F16, tag="wvt")
                    nc.sync.dma_start(
                        wvt[:], wvt_d[:].rearrange("(c p) e -> p c e", p=128)
                    )
                    for mt in range(SM):
                        v_ps = psA.tile([128, 512], F32, tag="pp", name="v_ps", bufs=4)[:, 0:EL]
                        for kc in range(KC):
                            nc.tensor.matmul(
                                v_ps[:],
                                xt[:, kc, ds(mt * 128, 128)],
                                wvt[:, kc, :],
                                start=(kc == 0), stop=(kc == KC - 1),
                            )
                        nc.vector.tensor_copy(
                            v[mt][:, :, 0:HD],
                            v_ps[:].rearrange("p (h d) -> p h d", h=HLOC),
                        )

            # ---- phase 2: attention, software-pipelined one unit deep ----
            with (
                tc.tile_pool(name="p2", bufs=2) as p2,
                tc.tile_pool(name="outp", bufs=1) as op,
                tc.tile_pool(name="ps_av", bufs=2, space="PSUM") as ps_av,
                tc.tile_pool(name="ps_f", bufs=2, space="PSUM") as ps_f,

            ):
                outT = [op.tile([HD, S], F16, tag=f"ot{h}", name=f"ot{h}")
                        for h in range(HLOC)]

                def stage_a(s1c, h):
                    ht, hb = h // 2, 64 * (h % 2)
                    s1_sl = ds(s1c * 512, 512)
                    halves = []
                    for hf in range(2):
                        blk = p2.tile([128, SM // 2, 512], F16, tag="exph",
                                      name="exph", bufs=8)
                        for s2p in range(SM // 4):
                            sc_ps = ps_sc.tile([128, 2, 512], F32, tag="sc",
                                               name="sc_ps")
                            for j in range(2):
                                s2m = hf * 8 + 2 * s2p + j
                                nc.tensor.matmul(
                                    sc_ps[:, j, :],
                                    kt[ht][hb:hb + 64, ds(s2m * 128, 128)],
                                    qt[ht][hb:hb + 64, s1_sl],
                                    start=True, stop=True,
                                )
                            nc.scalar.activation(
                                blk[:, ds(2 * s2p, 2), :], sc_ps[:], Exp,
                                scale=0.125
                            )
                        halves.append(blk)
                    return halves

                def stage_b(s1c, h, halves):
                    s1_sl = ds(s1c * 512, 512)
                    av_a = ps_av.tile([65, 512], F32, tag="ava", name="av_a")
                    av_b = ps_av.tile([65, 512], F32, tag="avb", name="av_b")
                    for s2m in range(SM):
                        dst = av_a if s2m < 8 else av_b
                        nc.tensor.matmul(
                            dst[:],
                            v[s2m][:, h, :],
                            halves[s2m // 8][:, s2m % 8, :],
                            start=(s2m % 8 == 0), stop=(s2m % 8 == 7),
                        )
                    av_cp = p2.tile([65, 512], F32, tag="avcp", name="av_cp")
                    nc.scalar.copy(av_cp[:], av_a[:])
                    av_sb = p2.tile([65, 512], F32, tag="avsb", name="av_sb")
                    nc.vector.tensor_add(av_sb[:], av_cp[:], av_b[:])
                    recip_sb = p2.tile([65, 512], F16, tag="recips", name="recip_sb")
                    with nc.allow_low_precision("fp16 softmax denominators"):
                        nc.vector.reciprocal(recip_sb[64:65, :], av_sb[64:65, :])
                    recip0 = p2.tile([1, 512], F16, tag="recip0", name="recip0")
                    nc.sync.dma_start(recip0[0:1, :], recip_sb[64:65, :])
                    rbc = p2.tile([128, 512], F16, tag="rbcsb", name="rbc_sb")
                    nc.gpsimd.partition_broadcast(rbc[:], recip0[0:1, :],
                                                  channels=128)
                    attn_r = attn_d[h].rearrange("(c p) s -> p c s", p=128)
                    for n4 in range(4):
                        blk = halves[n4 // 2]
                        c_sl = ds((n4 % 2) * 4, 4)
                        nc.vector.tensor_mul(
                            blk[:, c_sl, :], blk[:, c_sl, :],
                            rbc[:, None, :].broadcast_to([128, 4, 512]),
                        )
                        nc.sync.dma_start(
                            attn_r[:, ds(n4 * 4, 4), s1_sl], blk[:, c_sl, :]
                        )
                    nc.vector.tensor_mul(
                        outT[h][:, s1_sl], av_sb[0:64, :], rbc[0:64, :]
                    )

                def emit_wo(s1c):
                    for mt in range(4 * s1c, 4 * s1c + 4):
                        fst = p2.tile([128, D], F32, tag="fst", name="fst")
                        for nch in range(2):
                            f_ps = ps_f.tile([128, 512], F32, tag="fp", name="f_ps")
                            for h in range(HLOC):
                                nc.tensor.matmul(
                                    f_ps[:],
                                    outT[h][:, ds(mt * 128, 128)],
                                    woT[h][:, ds(nch * 512, 512)],
                                    start=(h == 0), stop=(h == HLOC - 1),
                                )
                            nc.scalar.copy(fst[:, ds(nch * 512, 512)], f_ps[:])
                        nc.sync.dma_start(outp_d[ds(mt * 128, 128), :], fst[:])

                units = [(s1c, h) for s1c in range(SC) for h in range(HLOC)]
                prev = None
                pending_wo = None
                for u in units:
                    blk = stage_a(*u)
                    if prev is not None:
                        stage_b(*prev)
                        if pending_wo is not None:
                            emit_wo(pending_wo)
                            pending_wo = None
                        if prev[1] == HLOC - 1:
                            pending_wo = prev[0]
                    prev = (u[0], u[1], blk)
                stage_b(*prev)
                if pending_wo is not None:
                    emit_wo(pending_wo)
                emit_wo(prev[0])

    nc.compile()
    return nc


def _prep_inputs(x, cos, sin, wq, wk, wv, wo):
    i_idx = (np.arange(128) % 64) // 2
    sgn = np.where(np.arange(128) % 2 == 0, -1.0, 1.0).astype(np.float32)[:, None]
    cos_rep = np.ascontiguousarray(cos.T[i_idx, :])
    sin_sgn = np.ascontiguousarray(sin.T[i_idx, :] * sgn)
    xts = [np.ascontiguousarray(x[b].T.astype(np.float16)) for b in range(B)]

    in_maps = []
    for c in range(NCORES):
        b, g = c // 4, c % 4
        rows = slice(g * EL, (g + 1) * EL)
        f16 = np.float16
        in_maps.append(dict(
            xt=xts[b],
            wqt=np.ascontiguousarray(wq[rows].T.astype(f16)),
            wkt=np.ascontiguousarray(wk[rows].T.astype(f16)),
            wvt=np.ascontiguousarray(wv[rows].T.astype(f16)),
            wot=np.ascontiguousarray(
                np.stack([wo[:, g * EL + h * HD: g * EL + (h + 1) * HD].T
                          for h in range(HLOC)]).astype(f16)
            ),
            cos=cos_rep,
            sin=sin_sgn,
        ))
    return in_maps


def kernel(x, cos, sin, wq, wk, wv, wo):
    global last_exec_time_ns
    x, cos, sin, wq, wk, wv, wo = [
        np.ascontiguousarray(np.asarray(a, dtype=np.float32))
        for a in (x, cos, sin, wq, wk, wv, wo)
    ]

    if "nc" not in _CACHE:
        _CACHE["nc"] = _build()
    nc = _CACHE["nc"]

    in_maps = _prep_inputs(x, cos, sin, wq, wk, wv, wo)
    trace = bool(int(os.environ.get("KERNEL_TRACE", "0")))
    r = run_bass_kernel_spmd(nc, in_maps, core_ids=list(range(NCORES)),
                             trace=trace)
    last_exec_time_ns = r.exec_time_ns

    out = np.zeros((B, S, D), dtype=np.float32)
    attn_w = np.empty((B, H, S, S), dtype=np.float32)
    for c in range(NCORES):
        b, g = c // 4, c % 4
        res = r.results[c]
        out[b] += res["out_p"]
        a = res["attn"]
        for j in range(HLOC):
            attn_w[b, g * HLOC + j] = a[j].T.astype(np.float32)
    return out, attn_w


# revision 19
# speedup vs baseline: 1.2890x; 1.2890x over previous
"""TRN2 Bass kernel for nn_Attention_25340307046737.

Multi-head attention (B=2, S=2048, D=1024, H=16, HD=64) with interleaved RoPE,
returning (out, attn_weights).  attn_weights is (2,16,2048,2048) f32 = 512MB,
so the kernel is HBM-write bound; everything is organized around streaming
those writes.

Sharding (8 NeuronCores): core c -> batch b = c//4, head group g = c%4
(4 heads each).  Column-parallel wq/wk/wv, row-parallel wo; the 4 partial
wo outputs per batch are summed on the host during unshard.

Per-core layout strategy:
  - x is passed transposed (D, S); q/k are computed directly in head-major
    (256, S) layout, v in seq-major (S, 256).
  - RoPE pair-swap is folded into the weights: a second projection with
    row-swapped wq/wk gives q_swap "for free" on the PE, then
    q_roped = q*cos_rep + q_swap*sin_sgn  (two DVE muls + add).
  - scores are computed transposed (s2 on partitions, s1 on free axis):
    lhsT = kT tile, rhs = qT tile.  exp via ACT (scale=1/8 folded in).
  - AV matmul uses a ones-augmented v (65 columns): row 64 of the PSUM
    accumulator is the softmax denominator, computed for free.
  - the attention loop is software-pipelined one (head, s1-chunk) unit
    deep: unit i's 16 score matmuls + exps run while unit i-1's AV
    matmuls run, so the PE never waits on ACT instruction-by-instruction
    (keeps the HAM clock gate open).
  - normalization: ACT reciprocal of the sums row, broadcast across
    partitions with a K=1 ones matmul, then 4 chunked in-place DVE
    multiplies each followed by its 1MB DMA of attn^T to HBM.

The host unshards: attn shard (4,2048,2048) holds attn^T per head, so the
host transposes per (b,h) 16MB block; wo partials are summed per batch.
"""
import os
import numpy as np

import concourse.bacc as bacc
import concourse.mybir as mybir
import concourse.tile as tile
from concourse.bass import ds
from concourse.bass_utils import run_bass_kernel_spmd

F32 = mybir.dt.float32
F32R = mybir.dt.float32r
F16 = mybir.dt.float16

B, S, D = 2, 2048, 1024
H, HD = 16, 64
HLOC = 4            # heads per core
EL = HLOC * HD      # 256 local output dims
NCORES = 8
KC = D // 128       # 8 contraction chunks
SC = S // 512       # 4 seq chunks of 512
SM = S // 128       # 16 seq chunks of 128

_CACHE = {}
last_exec_time_ns = None


def _build():
    nc = bacc.Bacc("TRN2", target_bir_lowering=False, debug=False,
                   num_devices=NCORES)

    xt_d = nc.dram_tensor("xt", [D, S], F16, kind="ExternalInput")
    wqt_d = nc.dram_tensor("wqt", [D, EL], F16, kind="ExternalInput")
    wkt_d = nc.dram_tensor("wkt", [D, EL], F16, kind="ExternalInput")
    wvt_d = nc.dram_tensor("wvt", [D, EL], F16, kind="ExternalInput")
    wot_d = nc.dram_tensor("wot", [HLOC, HD, D], F16, kind="ExternalInput")
    cos_d = nc.dram_tensor("cos", [128, S], F32, kind="ExternalInput")
    sin_d = nc.dram_tensor("sin", [128, S], F32, kind="ExternalInput")

    attn_d = nc.dram_tensor("attn", [HLOC, S, S], F16, kind="ExternalOutput")
    outp_d = nc.dram_tensor("out_p", [S, D], F32, kind="ExternalOutput")

    Exp = mybir.ActivationFunctionType.Exp

    with tile.TileContext(nc) as tc:
        with (
            tc.tile_pool(name="persist", bufs=1) as pp,
            tc.tile_pool(name="ps_sc", bufs=2, space="PSUM") as ps_sc,
        ):
            # ---- persistent tiles ----
            qt = [pp.tile([128, S], F16, tag=f"qt{i}", name=f"qt{i}") for i in range(2)]
            kt = [pp.tile([128, S], F16, tag=f"kt{i}", name=f"kt{i}") for i in range(2)]
            v = [pp.tile([128, HLOC, HD + 1], F16, tag=f"v{i}", name=f"v{i}") for i in range(SM)]
            woT = [pp.tile([HD, D], F16, tag=f"wo{h}", name=f"wo{h}") for h in range(HLOC)]
            ones_col = pp.tile([128, 1], F32, tag="ones_col")

            # ---- phase 1: projections + rope ----
            with (
                tc.tile_pool(name="p1", bufs=1) as p1,
                tc.tile_pool(name="psA", bufs=2, space="PSUM") as psA,
            ):
                xt = p1.tile([128, KC, S], F16, tag="xt")
                cos = p1.tile([128, S], F32, tag="cos")
                sin = p1.tile([128, S], F32, tag="sin")
                xt_r = xt_d[:].rearrange("(c p) s -> p c s", p=128)

                with (
                    tc.tile_pool(name="w1", bufs=1) as w1,
                    tc.tile_pool(name="tmp", bufs=2) as tp,
                ):
                    wsb = {}
                    for name in ("wqt", "wkt"):
                        wsb[name] = w1.tile([128, KC, EL], F16, tag=name, name=name)
                    # load order: q weights, first xt column block, then the
                    # rest -- lets the first matmuls start early.
                    nc.sync.dma_start(
                        wsb["wqt"][:], wqt_d[:].rearrange("(c p) e -> p c e", p=128))
                    for sc in range(SC):
                        s_sl = ds(sc * 512, 512)
                        for kc in range(KC):
                            nc.sync.dma_start(xt[:, kc, s_sl], xt_r[:, kc, s_sl])
                        if sc == 0:
                            nc.sync.dma_start(
                                wsb["wkt"][:],
                                wkt_d[:].rearrange("(c p) e -> p c e", p=128))
                            nc.sync.dma_start(cos[:], cos_d[:])
                            nc.sync.dma_start(sin[:], sin_d[:])
                    nc.vector.memset(ones_col[:], 1.0)
                    for i in range(SM):
                        nc.vector.tensor_copy(
                            v[i][:, :, HD:HD + 1],
                            ones_col[:].broadcast_to([128, HLOC, 1]),
                        )
                    for h in range(HLOC):
                        nc.sync.dma_start(woT[h][:], wot_d[h])

                    swap_mask = [i ^ 1 for i in range(32)]
                    for sc in range(SC):
                        s_sl = ds(sc * 512, 512)
                        for dest, wn in ((qt, "wqt"), (kt, "wkt")):
                            for mt in range(2):
                                q_ps = psA.tile([128, 512], F32, tag="pp", name="q_ps", bufs=4)
                                for kc in range(KC):
                                    nc.tensor.matmul(
                                        q_ps[:],
                                        wsb[wn][:, kc, ds(mt * 128, 128)],
                                        xt[:, kc, s_sl],
                                        start=(kc == 0), stop=(kc == KC - 1),
                                    )
                                qsw = tp.tile([128, 512], F32, tag="qsw", name="qsw")
                                nc.vector.stream_shuffle(qsw[:], q_ps[:], swap_mask)
                                t1 = tp.tile([128, 512], F32, tag="t1", name="t1")
                                t2 = tp.tile([128, 512], F32, tag="t2", name="t2")
                                nc.vector.tensor_mul(t1[:], q_ps[:], cos[:, s_sl])
                                nc.vector.tensor_mul(t2[:], qsw[:], sin[:, s_sl])
                                nc.vector.tensor_add(dest[mt][:, s_sl], t1[:], t2[:])

                with tc.tile_pool(name="w2", bufs=1) as w2:
                    wvt = w2.tile([128, KC, EL], F16, tag="wvt")
                    nc.sync.dma_start(
                        wvt[:], wvt_d[:].rearrange("(c p) e -> p c e", p=128)
                    )
                    for mt in range(SM):
                        v_ps = psA.tile([128, 512], F32, tag="pp", name="v_ps", bufs=4)[:, 0:EL]
                        for kc in range(KC):
                            nc.tensor.matmul(
                                v_ps[:],
                                xt[:, kc, ds(mt * 128, 128)],
                                wvt[:, kc, :],
                                start=(kc == 0), stop=(kc == KC - 1),
                            )
                        nc.vector.tensor_copy(
                            v[mt][:, :, 0:HD],
                            v_ps[:].rearrange("p (h d) -> p h d", h=HLOC),
                        )

            # ---- phase 2: attention, software-pipelined one unit deep ----
            with (
                tc.tile_pool(name="p2", bufs=2) as p2,
                tc.tile_pool(name="outp", bufs=1) as op,
                tc.tile_pool(name="ps_av", bufs=2, space="PSUM") as ps_av,
                tc.tile_pool(name="ps_f", bufs=2, space="PSUM") as ps_f,

            ):
                outT = [op.tile([HD, S], F16, tag=f"ot{h}", name=f"ot{h}")
                        for h in range(HLOC)]

                def stage_a(s1c, h):
                    ht, hb = h // 2, 64 * (h % 2)
                    s1_sl = ds(s1c * 512, 512)
                    halves = []
                    for hf in range(2):
                        blk = p2.tile([128, SM // 2, 512], F16, tag="exph",
                                      name="exph", bufs=8)
                        for s2p in range(SM // 4):
                            sc_ps = ps_sc.tile([128, 2, 512], F32, tag="sc",
                                               name="sc_ps")
                            for j in range(2):
                                s2m = hf * 8 + 2 * s2p + j
                                nc.tensor.matmul(
                                    sc_ps[:, j, :],
                                    kt[ht][hb:hb + 64, ds(s2m * 128, 128)],
                                    qt[ht][hb:hb + 64, s1_sl],
                                    start=True, stop=True,
                                )
                            nc.scalar.activation(
                                blk[:, ds(2 * s2p, 2), :], sc_ps[:], Exp,
                                scale=0.125
                            )
                        halves.append(blk)
                    return halves

                def stage_b(s1c, h, halves):
                    s1_sl = ds(s1c * 512, 512)
                    av_a = ps_av.tile([65, 512], F32, tag="av", name="av_a")
                    av_b = ps_av.tile([65, 512], F32, tag="av", name="av_b")
                    for s2m in range(SM):
                        dst = av_a if s2m < 8 else av_b
                        nc.tensor.matmul(
                            dst[:],
                            v[s2m][:, h, :],
                            halves[s2m // 8][:, s2m % 8, :],
                            start=(s2m % 8 == 0), stop=(s2m % 8 == 7),
                        )
                    av_cp = p2.tile([65, 512], F32, tag="avcp", name="av_cp")
                    nc.scalar.copy(av_cp[:], av_a[:])
                    av_sb = p2.tile([65, 512], F32, tag="avsb", name="av_sb")
                    nc.vector.tensor_add(av_sb[:], av_cp[:], av_b[:])
                    recip_sb = p2.tile([65, 512], F16, tag="recips", name="recip_sb")
                    with nc.allow_low_precision("fp16 softmax denominators"):
                        nc.vector.reciprocal(recip_sb[64:65, :], av_sb[64:65, :])
                    recip0 = p2.tile([1, 512], F16, tag="recip0", name="recip0")
                    nc.sync.dma_start(recip0[0:1, :], recip_sb[64:65, :])
                    rbc = p2.tile([128, 512], F16, tag="rbcsb", name="rbc_sb")
                    nc.gpsimd.partition_broadcast(rbc[:], recip0[0:1, :],
                                                  channels=128)
                    attn_r = attn_d[h].rearrange("(c p) s -> p c s", p=128)
                    for n4 in range(4):
                        blk = halves[n4 // 2]
                        c_sl = ds((n4 % 2) * 4, 4)
                        nc.vector.tensor_mul(
                            blk[:, c_sl, :], blk[:, c_sl, :],
                            rbc[:, None, :].broadcast_to([128, 4, 512]),
                        )
                        nc.sync.dma_start(
                            attn_r[:, ds(n4 * 4, 4), s1_sl], blk[:, c_sl, :]
                        )
                    nc.vector.tensor_mul(
                        outT[h][:, s1_sl], av_sb[0:64, :], rbc[0:64, :]
                    )

                def emit_wo(s1c):
                    for mt in range(4 * s1c, 4 * s1c + 4):
                        fst = p2.tile([128, D], F32, tag="fst", name="fst")
                        for nch in range(2):
                            f_ps = ps_f.tile([128, 512], F32, tag="fp", name="f_ps")
                            for h in range(HLOC):
                                nc.tensor.matmul(
                                    f_ps[:],
                                    outT[h][:, ds(mt * 128, 128)],
                                    woT[h][:, ds(nch * 512, 512)],
                                    start=(h == 0), stop=(h == HLOC - 1),
                                )
                            nc.scalar.copy(fst[:, ds(nch * 512, 512)], f_ps[:])
                        nc.sync.dma_start(outp_d[ds(mt * 128, 128), :], fst[:])

                units = [(s1c, h) for s1c in range(SC) for h in range(HLOC)]
                prev = None
                pending_wo = None
                for u in units:
                    blk = stage_a(*u)
                    if prev is not None:
                        stage_b(*prev)
                        if pending_wo is not None:
                            emit_wo(pending_wo)
                            pending_wo = None
                        if prev[1] == HLOC - 1:
                            pending_wo = prev[0]
                    prev = (u[0], u[1], blk)
                stage_b(*prev)
                if pending_wo is not None:
                    emit_wo(pending_wo)
                emit_wo(prev[0])

    nc.compile()
    return nc


def _prep_inputs(x, cos, sin, wq, wk, wv, wo):
    i_idx = (np.arange(128) % 64) // 2
    sgn = np.where(np.arange(128) % 2 == 0, -1.0, 1.0).astype(np.float32)[:, None]
    cos_rep = np.ascontiguousarray(cos.T[i_idx, :])
    sin_sgn = np.ascontiguousarray(sin.T[i_idx, :] * sgn)
    xts = [np.ascontiguousarray(x[b].T.astype(np.float16)) for b in range(B)]

    in_maps = []
    for c in range(NCORES):
        b, g = c // 4, c % 4
        rows = slice(g * EL, (g + 1) * EL)
        f16 = np.float16
        in_maps.append(dict(
            xt=xts[b],
            wqt=np.ascontiguousarray(wq[rows].T.astype(f16)),
            wkt=np.ascontiguousarray(wk[rows].T.astype(f16)),
            wvt=np.ascontiguousarray(wv[rows].T.astype(f16)),
            wot=np.ascontiguousarray(
                np.stack([wo[:, g * EL + h * HD: g * EL + (h + 1) * HD].T
                          for h in range(HLOC)]).astype(f16)
            ),
            cos=cos_rep,
            sin=sin_sgn,
        ))
    return in_maps


def kernel(x, cos, sin, wq, wk, wv, wo):
    global last_exec_time_ns
    x, cos, sin, wq, wk, wv, wo = [
        np.ascontiguousarray(np.asarray(a, dtype=np.float32))
        for a in (x, cos, sin, wq, wk, wv, wo)
    ]

    if "nc" not in _CACHE:
        _CACHE["nc"] = _build()
    nc = _CACHE["nc"]

    in_maps = _prep_inputs(x, cos, sin, wq, wk, wv, wo)
    trace = bool(int(os.environ.get("KERNEL_TRACE", "0")))
    r = run_bass_kernel_spmd(nc, in_maps, core_ids=list(range(NCORES)),
                             trace=trace)
    last_exec_time_ns = r.exec_time_ns

    out = np.zeros((B, S, D), dtype=np.float32)
    attn_w = np.empty((B, H, S, S), dtype=np.float32)
    for c in range(NCORES):
        b, g = c // 4, c % 4
        res = r.results[c]
        out[b] += res["out_p"]
        a = res["attn"]
        for j in range(HLOC):
            attn_w[b, g * HLOC + j] = a[j].T.astype(np.float32)
    return out, attn_w


# revision 20
# speedup vs baseline: 1.3769x; 1.0682x over previous
"""TRN2 Bass kernel for nn_Attention_25340307046737.

Multi-head attention (B=2, S=2048, D=1024, H=16, HD=64) with interleaved RoPE,
returning (out, attn_weights).  attn_weights is (2,16,2048,2048) f32 = 512MB,
so the kernel is HBM-write bound; everything is organized around streaming
those writes.

Sharding (8 NeuronCores): core c -> batch b = c//4, head group g = c%4
(4 heads each).  Column-parallel wq/wk/wv, row-parallel wo; the 4 partial
wo outputs per batch are summed on the host during unshard.

Per-core layout strategy:
  - x is passed transposed (D, S); q/k are computed directly in head-major
    (256, S) layout, v in seq-major (S, 256).
  - RoPE pair-swap is folded into the weights: a second projection with
    row-swapped wq/wk gives q_swap "for free" on the PE, then
    q_roped = q*cos_rep + q_swap*sin_sgn  (two DVE muls + add).
  - scores are computed transposed (s2 on partitions, s1 on free axis):
    lhsT = kT tile, rhs = qT tile.  exp via ACT (scale=1/8 folded in).
  - AV matmul uses a ones-augmented v (65 columns): row 64 of the PSUM
    accumulator is the softmax denominator, computed for free.
  - the attention loop is software-pipelined one (head, s1-chunk) unit
    deep: unit i's 16 score matmuls + exps run while unit i-1's AV
    matmuls run, so the PE never waits on ACT instruction-by-instruction
    (keeps the HAM clock gate open).
  - normalization: ACT reciprocal of the sums row, broadcast across
    partitions with a K=1 ones matmul, then 4 chunked in-place DVE
    multiplies each followed by its 1MB DMA of attn^T to HBM.

The host unshards: attn shard (4,2048,2048) holds attn^T per head, so the
host transposes per (b,h) 16MB block; wo partials are summed per batch.
"""
import os
import numpy as np

import concourse.bacc as bacc
import concourse.mybir as mybir
import concourse.tile as tile
from concourse.bass import ds
from concourse.bass_utils import run_bass_kernel_spmd

F32 = mybir.dt.float32
F32R = mybir.dt.float32r
F16 = mybir.dt.float16

B, S, D = 2, 2048, 1024
H, HD = 16, 64
HLOC = 4            # heads per core
EL = HLOC * HD      # 256 local output dims
NCORES = 8
KC = D // 128       # 8 contraction chunks
SC = S // 512       # 4 seq chunks of 512
SM = S // 128       # 16 seq chunks of 128

_CACHE = {}
last_exec_time_ns = None


def _build():
    nc = bacc.Bacc("TRN2", target_bir_lowering=False, debug=False,
                   num_devices=NCORES)

    xt_d = nc.dram_tensor("xt", [D, S], F16, kind="ExternalInput")
    wqt_d = nc.dram_tensor("wqt", [D, EL], F16, kind="ExternalInput")
    wkt_d = nc.dram_tensor("wkt", [D, EL], F16, kind="ExternalInput")
    wvt_d = nc.dram_tensor("wvt", [D, EL], F16, kind="ExternalInput")
    wot_d = nc.dram_tensor("wot", [HLOC, HD, D], F16, kind="ExternalInput")
    cos_d = nc.dram_tensor("cos", [128, S], F32, kind="ExternalInput")
    sin_d = nc.dram_tensor("sin", [128, S], F32, kind="ExternalInput")

    attn_d = nc.dram_tensor("attn", [HLOC, S, S], F16, kind="ExternalOutput")
    outp_d = nc.dram_tensor("out_p", [S, D], F32, kind="ExternalOutput")

    Exp = mybir.ActivationFunctionType.Exp

    with tile.TileContext(nc) as tc:
        with (
            tc.tile_pool(name="persist", bufs=1) as pp,
            tc.tile_pool(name="ps_sc", bufs=2, space="PSUM") as ps_sc,
        ):
            # ---- persistent tiles ----
            qt = [pp.tile([128, S], F16, tag=f"qt{i}", name=f"qt{i}") for i in range(2)]
            kt = [pp.tile([128, S], F16, tag=f"kt{i}", name=f"kt{i}") for i in range(2)]
            v = [pp.tile([128, HLOC, HD + 1], F16, tag=f"v{i}", name=f"v{i}") for i in range(SM)]
            woT = [pp.tile([HD, D], F16, tag=f"wo{h}", name=f"wo{h}") for h in range(HLOC)]
            ones_col = pp.tile([128, 1], F32, tag="ones_col")

            # ---- phase 1: projections + rope ----
            with (
                tc.tile_pool(name="p1", bufs=1) as p1,
                tc.tile_pool(name="psA", bufs=2, space="PSUM") as psA,
            ):
                xt = p1.tile([128, KC, S], F16, tag="xt")
                cos = p1.tile([128, S], F32, tag="cos")
                sin = p1.tile([128, S], F32, tag="sin")
                xt_r = xt_d[:].rearrange("(c p) s -> p c s", p=128)

                with (
                    tc.tile_pool(name="w1", bufs=1) as w1,
                    tc.tile_pool(name="tmp", bufs=2) as tp,
                ):
                    wsb = {}
                    for name in ("wqt", "wkt"):
                        wsb[name] = w1.tile([128, KC, EL], F16, tag=name, name=name)
                    # load order: q weights, first xt column block, then the
                    # rest -- lets the first matmuls start early.
                    nc.sync.dma_start(
                        wsb["wqt"][:], wqt_d[:].rearrange("(c p) e -> p c e", p=128))
                    for sc in range(SC):
                        s_sl = ds(sc * 512, 512)
                        for kc in range(KC):
                            nc.sync.dma_start(xt[:, kc, s_sl], xt_r[:, kc, s_sl])
                        if sc == 0:
                            nc.sync.dma_start(
                                wsb["wkt"][:],
                                wkt_d[:].rearrange("(c p) e -> p c e", p=128))
                            nc.sync.dma_start(cos[:], cos_d[:])
                            nc.sync.dma_start(sin[:], sin_d[:])
                    nc.vector.memset(ones_col[:], 1.0)
                    for i in range(SM):
                        nc.vector.tensor_copy(
                            v[i][:, :, HD:HD + 1],
                            ones_col[:].broadcast_to([128, HLOC, 1]),
                        )
                    for h in range(HLOC):
                        nc.sync.dma_start(woT[h][:], wot_d[h])

                    swap_mask = [i ^ 1 for i in range(32)]
                    for sc in range(SC):
                        s_sl = ds(sc * 512, 512)
                        for dest, wn in ((qt, "wqt"), (kt, "wkt")):
                            for mt in range(2):
                                q_ps = psA.tile([128, 512], F32, tag="pp", name="q_ps", bufs=4)
                                for kc in range(KC):
                                    nc.tensor.matmul(
                                        q_ps[:],
                                        wsb[wn][:, kc, ds(mt * 128, 128)],
                                        xt[:, kc, s_sl],
                                        start=(kc == 0), stop=(kc == KC - 1),
                                    )
                                qsw = tp.tile([128, 512], F32, tag="qsw", name="qsw")
                                nc.vector.stream_shuffle(qsw[:], q_ps[:], swap_mask)
                                t1 = tp.tile([128, 512], F32, tag="t1", name="t1")
                                t2 = tp.tile([128, 512], F32, tag="t2", name="t2")
                                nc.vector.tensor_mul(t1[:], q_ps[:], cos[:, s_sl])
                                nc.vector.tensor_mul(t2[:], qsw[:], sin[:, s_sl])
                                nc.vector.tensor_add(dest[mt][:, s_sl], t1[:], t2[:])

                with tc.tile_pool(name="w2", bufs=1) as w2:
                    wvt = w2.tile([128, KC, EL], F16, tag="wvt")
                    nc.sync.dma_start(
                        wvt[:], wvt_d[:].rearrange("(c p) e -> p c e", p=128)
                    )
                    for mt in range(SM):
                        v_ps = psA.tile([128, 512], F32, tag="pp", name="v_ps", bufs=4)[:, 0:EL]
                        for kc in range(KC):
                            nc.tensor.matmul(
                                v_ps[:],
                                xt[:, kc, ds(mt * 128, 128)],
                                wvt[:, kc, :],
                                start=(kc == 0), stop=(kc == KC - 1),
                            )
                        nc.vector.tensor_copy(
                            v[mt][:, :, 0:HD],
                            v_ps[:].rearrange("p (h d) -> p h d", h=HLOC),
                        )

            # ---- phase 2: attention, software-pipelined one unit deep ----
            with (
                tc.tile_pool(name="p2", bufs=2) as p2,
                tc.tile_pool(name="outp", bufs=1) as op,
                tc.tile_pool(name="ps_av", bufs=2, space="PSUM") as ps_av,
                tc.tile_pool(name="ps_f", bufs=2, space="PSUM") as ps_f,

            ):
                outT = [op.tile([HD, S], F16, tag=f"ot{h}", name=f"ot{h}")
                        for h in range(HLOC)]

                def stage_a(s1c, h):
                    ht, hb = h // 2, 64 * (h % 2)
                    s1_sl = ds(s1c * 512, 512)
                    halves = []
                    for hf in range(2):
                        blk = p2.tile([128, SM // 2, 512], F16, tag="exph",
                                      name="exph", bufs=8)
                        for s2p in range(SM // 4):
                            sc_ps = ps_sc.tile([128, 2, 512], F32, tag="sc",
                                               name="sc_ps")
                            for j in range(2):
                                s2m = hf * 8 + 2 * s2p + j
                                nc.tensor.matmul(
                                    sc_ps[:, j, :],
                                    kt[ht][hb:hb + 64, ds(s2m * 128, 128)],
                                    qt[ht][hb:hb + 64, s1_sl],
                                    start=True, stop=True,
                                )
                            nc.scalar.activation(
                                blk[:, ds(2 * s2p, 2), :], sc_ps[:], Exp,
                                scale=0.125
                            )
                        halves.append(blk)
                    return halves

                def stage_b(s1c, h, halves):
                    s1_sl = ds(s1c * 512, 512)
                    av_a = ps_av.tile([65, 512], F32, tag="av", name="av_a")
                    av_b = ps_av.tile([65, 512], F32, tag="av", name="av_b")
                    for s2m in range(SM):
                        dst = av_a if s2m < 8 else av_b
                        nc.tensor.matmul(
                            dst[:],
                            v[s2m][:, h, :],
                            halves[s2m // 8][:, s2m % 8, :],
                            start=(s2m % 8 == 0), stop=(s2m % 8 == 7),
                        )
                    av_cp = p2.tile([65, 512], F32, tag="avcp", name="av_cp")
                    nc.scalar.copy(av_cp[:], av_a[:])
                    av_sb = p2.tile([65, 512], F32, tag="avsb", name="av_sb")
                    nc.vector.tensor_add(av_sb[:], av_cp[:], av_b[:])
                    recip_sb = p2.tile([65, 512], F16, tag="recips", name="recip_sb")
                    with nc.allow_low_precision("fp16 softmax denominators"):
                        nc.vector.reciprocal(recip_sb[64:65, :], av_sb[64:65, :])
                    recip0 = p2.tile([1, 512], F16, tag="recip0", name="recip0")
                    nc.sync.dma_start(recip0[0:1, :], recip_sb[64:65, :])
                    rbc = p2.tile([128, 512], F16, tag="rbcsb", name="rbc_sb")
                    nc.gpsimd.partition_broadcast(rbc[:], recip0[0:1, :],
                                                  channels=128)
                    attn_r = attn_d[h].rearrange("(c p) s -> p c s", p=128)
                    for n4 in range(4):
                        blk = halves[n4 // 2]
                        c_sl = ds((n4 % 2) * 4, 4)
                        nc.vector.tensor_mul(
                            blk[:, c_sl, :], blk[:, c_sl, :],
                            rbc[:, None, :].broadcast_to([128, 4, 512]),
                        )
                        nc.sync.dma_start(
                            attn_r[:, ds(n4 * 4, 4), s1_sl], blk[:, c_sl, :]
                        )
                    nc.vector.tensor_mul(
                        outT[h][:, s1_sl], av_sb[0:64, :], rbc[0:64, :]
                    )

                def emit_wo_mt(mt):
                    fst = p2.tile([128, D], F32, tag="fst", name="fst")
                    for nch in range(2):
                        f_ps = ps_f.tile([128, 512], F32, tag="fp", name="f_ps")
                        for h in range(HLOC):
                            nc.tensor.matmul(
                                f_ps[:],
                                outT[h][:, ds(mt * 128, 128)],
                                woT[h][:, ds(nch * 512, 512)],
                                start=(h == 0), stop=(h == HLOC - 1),
                            )
                        nc.scalar.copy(fst[:, ds(nch * 512, 512)], f_ps[:])
                    nc.sync.dma_start(outp_d[ds(mt * 128, 128), :], fst[:])

                units = [(s1c, h) for s1c in range(SC) for h in range(HLOC)]
                prev = None
                wo_queue = []
                for u in units:
                    blk = stage_a(*u)
                    if prev is not None:
                        stage_b(*prev)
                        if wo_queue:
                            emit_wo_mt(wo_queue.pop(0))
                        if prev[1] == HLOC - 1:
                            wo_queue.extend(range(4 * prev[0], 4 * prev[0] + 4))
                    prev = (u[0], u[1], blk)
                stage_b(*prev)
                wo_queue.extend(range(4 * prev[0], 4 * prev[0] + 4))
                for mt in wo_queue:
                    emit_wo_mt(mt)

    nc.compile()
    return nc


def _prep_inputs(x, cos, sin, wq, wk, wv, wo):
    i_idx = (np.arange(128) % 64) // 2
    sgn = np.where(np.arange(128) % 2 == 0, -1.0, 1.0).astype(np.float32)[:, None]
    cos_rep = np.ascontiguousarray(cos.T[i_idx, :])
    sin_sgn = np.ascontiguousarray(sin.T[i_idx, :] * sgn)
    xts = [np.ascontiguousarray(x[b].T.astype(np.float16)) for b in range(B)]

    in_maps = []
    for c in range(NCORES):
        b, g = c // 4, c % 4
        rows = slice(g * EL, (g + 1) * EL)
        f16 = np.float16
        in_maps.append(dict(
            xt=xts[b],
            wqt=np.ascontiguousarray(wq[rows].T.astype(f16)),
            wkt=np.ascontiguousarray(wk[rows].T.astype(f16)),
            wvt=np.ascontiguousarray(wv[rows].T.astype(f16)),
            wot=np.ascontiguousarray(
                np.stack([wo[:, g * EL + h * HD: g * EL + (h + 1) * HD].T
                          for h in range(HLOC)]).astype(f16)
            ),
            cos=cos_rep,
            sin=sin_sgn,
        ))
    return in_maps


def kernel(x, cos, sin, wq, wk, wv, wo):
    global last_exec_time_ns
    x, cos, sin, wq, wk, wv, wo = [
        np.ascontiguousarray(np.asarray(a, dtype=np.float32))
        for a in (x, cos, sin, wq, wk, wv, wo)
    ]

    if "nc" not in _CACHE:
        _CACHE["nc"] = _build()
    nc = _CACHE["nc"]

    in_maps = _prep_inputs(x, cos, sin, wq, wk, wv, wo)
    trace = bool(int(os.environ.get("KERNEL_TRACE", "0")))
    r = run_bass_kernel_spmd(nc, in_maps, core_ids=list(range(NCORES)),
                             trace=trace)
    last_exec_time_ns = r.exec_time_ns

    out = np.zeros((B, S, D), dtype=np.float32)
    attn_w = np.empty((B, H, S, S), dtype=np.float32)
    for c in range(NCORES):
        b, g = c // 4, c % 4
        res = r.results[c]
        out[b] += res["out_p"]
        a = res["attn"]
        for j in range(HLOC):
            attn_w[b, g * HLOC + j] = a[j].T.astype(np.float32)
    return out, attn_w
